# revision 8
# baseline (speedup 1.0000x reference)
"""FFTConv2d kernel for trn2, 8 NeuronCores.

Math: reference einsum 'bchw,oihw->bohw' factorizes:
  Y[b,o] = conv_full(sum_c x[b,c], sum_i w[o,i])[1:-1,1:-1] + bias[o]
i.e. a single-channel 3x3 "same" convolution (flipped kernel) per (b,o).

Host marshaling per core (2 batches): channel-sum xs = sum_c x (linear,
exact fp32), zero-pad, and materialize the 9 shifted tap windows as rows
of a [19, 128*128] bf16 matrix (2 batches x 9 taps + ones row for bias).
Every column is an independent output pixel, so there are no pad columns
anywhere on device.  K-rows 0-15 ship as xp9a [128, 2048] (a layout all
16 SDMA engines load in parallel; [19, n] loads land on one engine) and
are reshaped on-chip back to [16, 16384] per slice via SBUF->SBUF DMA;
K-rows 16-18 ride 4 rotating gpsimd loads.

Device per core:
  1. Load wb + stagA + xp9b rows; 8 warm-up matmuls on wb keep the PE
     clock ramped while inputs land.
  2. Conv: per 4-row chunk, one K=19 bf16 matmul wb^T @ xin[:, 512-col
     chunk] -> one full PSUM bank [128, 512] (all (b,o) at once, bias
     rides the ones row).  Two chunks fill a 2-bank PSUM tile.
  3. One contiguous rank-2 copy [128, 1024] f32->f16 per PSUM tile
     (8 output rows), alternating vector/scalar engines.
  4. Store yt -> HBM fp16 per 8 output rows, alternating sync/gpsimd.
Host casts the fp16 result back to fp32.
"""

import os
import sys
from functools import lru_cache

import numpy as np

for _p in ("/opt/trn_rl_repo", "/root/.axon_site/_ro/trn_rl_repo"):
    if os.path.isdir(_p) and _p not in sys.path:
        sys.path.insert(0, _p)

import ml_dtypes

B, CIN, COUT, H, W = 16, 64, 64, 128, 128
N_CORES = 8
BPC = B // N_CORES  # batches per core = 2
NOUT = BPC * COUT  # 128 output partitions (b, o)
KP = BPC * 9 + 1  # 19 matmul K partitions (b, tap) + ones
NCOLS = H * W  # xin free length = 16384 (no pad columns)
NS = 4  # input slices
SLICE_COLS = NCOLS // NS  # 4096
RCOL = SLICE_COLS // 8  # 512, reshape src cols per slice
CHW = 4 * W  # matmul chunk = 4 output rows = 512 cols = 1 PSUM bank
NCHUNK = NCOLS // CHW  # 32
NWARM = 8


@lru_cache(maxsize=1)
def _build():
    import concourse.bacc as bacc
    import concourse.mybir as mybir
    import concourse.tile as tile

    f32 = mybir.dt.float32
    bf16 = mybir.dt.bfloat16
    f16 = mybir.dt.float16

    nc = bacc.Bacc("TRN2", target_bir_lowering=False, debug=False, num_devices=N_CORES)

    xp9a = nc.dram_tensor("xp9a", [128, NS * RCOL], bf16, kind="ExternalInput")
    xp9b = nc.dram_tensor("xp9b", [3, NCOLS], bf16, kind="ExternalInput")
    wb = nc.dram_tensor("wb", [KP, NOUT], bf16, kind="ExternalInput")
    y = nc.dram_tensor("y", [NOUT, H * W], f16, kind="ExternalOutput")

    with tile.TileContext(nc) as tc:
        with (
            tc.tile_pool(name="xin", bufs=1) as xin_pool,
            tc.tile_pool(name="stag", bufs=1) as stag_pool,
            tc.tile_pool(name="yout", bufs=1) as y_pool,
            tc.tile_pool(name="consts", bufs=1) as c_pool,
            tc.tile_pool(name="cv_ps", bufs=4, space="PSUM") as cv_psum,
        ):
            wb_t = c_pool.tile([KP, NOUT], bf16, tag="wb")
            nc.sync.dma_start(out=wb_t[:, :], in_=wb.ap()[:, :])

            stagA = stag_pool.tile([128, NS * RCOL], bf16, tag="stagA")
            nc.scalar.dma_start(out=stagA[:, :], in_=xp9a.ap()[:, :])

            xin = xin_pool.tile([KP, NCOLS], bf16, tag="xin")
            # K-rows 16-18 (last 2 taps + ones): direct loads, rotating
            # SWDGE engines
            for s in range(NS):
                c0 = s * SLICE_COLS
                nc.gpsimd.dma_start(
                    out=xin[16:19, c0 : c0 + SLICE_COLS],
                    in_=xp9b.ap()[:, c0 : c0 + SLICE_COLS],
                )
            # warm-up matmuls: ramp the PE clock while inputs land; they
            # scribble on cv-pool tiles that real matmuls overwrite
            wtiles = [
                cv_psum.tile([NOUT, 1024], f32, tag="cv", name=f"warm{i}")
                for i in range(4)
            ]
            for wi in range(NWARM):
                nc.tensor.matmul(
                    wtiles[wi % 4][:, 0:128],
                    wb_t[:, :],
                    wb_t[:, :],
                    start=True,
                    stop=True,
                )
            # K-rows 0-15: per-slice SBUF->SBUF reshape from stagA
            for s in range(NS):
                nc.sync.dma_start(
                    out=xin[0:16, s * SLICE_COLS : (s + 1) * SLICE_COLS],
                    in_=stagA[:, s * RCOL : (s + 1) * RCOL],
                )

            yt = y_pool.tile([NOUT, NCOLS], f16, tag="yt")

            def cp_vec(dst, src):
                nc.vector.tensor_copy(dst, src)

            def cp_act(dst, src):
                nc.scalar.copy(dst, src)

            cpe = [cp_vec, cp_act]
            dmae = [nc.sync, nc.gpsimd]
            for j in range(NCHUNK // 2):  # 16 copy/store groups of 8 rows
                ps = cv_psum.tile([NOUT, 1024], f32, tag="cv")
                for h in range(2):
                    u0 = (2 * j + h) * CHW
                    nc.tensor.matmul(
                        ps[:, h * CHW : (h + 1) * CHW],
                        wb_t[:, :],
                        xin[:, u0 : u0 + CHW],
                        start=True,
                        stop=True,
                    )
                c0 = j * 2 * CHW
                cpe[j % 2](yt[:, c0 : c0 + 2 * CHW], ps[:, :])
                dmae[j % 2].dma_start(
                    out=y.ap()[:, c0 : c0 + 2 * CHW],
                    in_=yt[:, c0 : c0 + 2 * CHW],
                )

    nc.compile()
    return nc


def _host_prep(x, weight, bias):
    bf = ml_dtypes.bfloat16
    wsum = weight.sum(axis=1)  # [COUT, 3, 3]
    wb = np.zeros((KP, NOUT), np.float32)
    for b in range(BPC):
        for di in range(3):
            for dj in range(3):
                wb[b * 9 + di * 3 + dj, b * COUT : (b + 1) * COUT] = wsum[
                    :, 2 - di, 2 - dj
                ]
    wb[KP - 1, :] = np.tile(bias, BPC)
    wb = wb.astype(bf)

    in_maps = []
    for r in range(N_CORES):
        xs = x[r * BPC : (r + 1) * BPC].sum(axis=1)  # [BPC, H, W] fp32
        xpad = np.zeros((BPC, H + 2, W + 2), np.float32)
        xpad[:, 1 : H + 1, 1 : W + 1] = xs
        xpad = xpad.astype(bf)
        xp9 = np.empty((KP, NCOLS), bf)
        for di in range(3):
            for dj in range(3):
                m = di * 3 + dj
                win = xpad[:, di : di + H, dj : dj + W]  # [BPC, H, W]
                for b in range(BPC):
                    xp9[b * 9 + m] = win[b].reshape(NCOLS)
        xp9[KP - 1] = np.ones((NCOLS,), np.float32).astype(bf)
        # K-rows 0-15 packed for the [128, 2048] spread-load + per-slice
        # on-chip reshape: xp9a[8p+g, s*512+c] = xp9[p, s*4096+g*512+c]
        xp9a = np.ascontiguousarray(
            xp9[0:16].reshape(16, NS, 8, RCOL).transpose(0, 2, 1, 3)
        ).reshape(128, NS * RCOL)
        xp9b = np.ascontiguousarray(xp9[16:19])
        in_maps.append({"xp9a": xp9a, "xp9b": xp9b, "wb": wb})
    return in_maps


def kernel(x, weight, bias):
    from concourse.bass_utils import run_bass_kernel_spmd

    x = np.asarray(x, dtype=np.float32)
    weight = np.asarray(weight, dtype=np.float32)
    bias = np.asarray(bias, dtype=np.float32)
    nc = _build()
    in_maps = _host_prep(x, weight, bias)
    res = run_bass_kernel_spmd(nc, in_maps, core_ids=list(range(N_CORES)))
    out = np.concatenate(
        [
            np.asarray(res.results[r]["y"]).reshape(BPC, COUT, H, W)
            for r in range(N_CORES)
        ],
        axis=0,
    )
    return out.astype(np.float32)


# revision 13
# speedup vs baseline: 1.0598x; 1.0598x over previous
"""FFTConv2d kernel for trn2, 8 NeuronCores.

Math: reference einsum 'bchw,oihw->bohw' factorizes:
  Y[b,o] = conv_full(sum_c x[b,c], sum_i w[o,i])[1:-1,1:-1] + bias[o]
i.e. a single-channel 3x3 "same" convolution (flipped kernel) per (b,o).

Host marshaling per core (2 batches): channel-sum xs = sum_c x (linear,
exact fp32), zero-pad, and materialize the 9 shifted tap windows as rows
of a [19, 128*128] bf16 matrix (2 batches x 9 taps + ones row for bias).
Every column is an independent output pixel, so there are no pad columns
anywhere on device.  K-rows 0-15 ship as xp9a [128, 2048] (a layout all
16 SDMA engines load in parallel; [19, n] loads land on one engine) and
are reshaped on-chip back to [16, 16384] per slice via SBUF->SBUF DMA;
K-rows 16-18 ride 4 rotating gpsimd loads.

Device per core:
  1. Load wb + stagA + xp9b rows; 8 warm-up matmuls on wb keep the PE
     clock ramped while inputs land.
  2. Conv: per 4-row chunk, one K=19 bf16 matmul wb^T @ xin[:, 512-col
     chunk] -> one full PSUM bank [128, 512] (all (b,o) at once, bias
     rides the ones row).  Two chunks fill a 2-bank PSUM tile.
  3. One contiguous rank-2 copy [128, 1024] f32->f16 per PSUM tile
     (8 output rows), alternating vector/scalar engines.
  4. Store yt -> HBM fp16 per 8 output rows, alternating sync/gpsimd.
Host casts the fp16 result back to fp32.
"""

import os
import sys
from functools import lru_cache

import numpy as np

for _p in ("/opt/trn_rl_repo", "/root/.axon_site/_ro/trn_rl_repo"):
    if os.path.isdir(_p) and _p not in sys.path:
        sys.path.insert(0, _p)

import ml_dtypes

B, CIN, COUT, H, W = 16, 64, 64, 128, 128
N_CORES = 8
BPC = B // N_CORES  # batches per core = 2
NOUT = BPC * COUT  # 128 output partitions (b, o)
KP = BPC * 9 + 1  # 19 matmul K partitions (b, tap) + ones
NCOLS = H * W  # xin free length = 16384 (no pad columns)
NS = 4  # input slices
SLICE_COLS = NCOLS // NS  # 4096
RCOL = SLICE_COLS // 8  # 512, reshape src cols per slice
CHW = 4 * W  # matmul chunk = 4 output rows = 512 cols = 1 PSUM bank
NCHUNK = NCOLS // CHW  # 32
NWARM = 11
# copy/store groups in units of 512-col banks: two small groups up front
# so the first store fires early, then 2-bank groups
_GROUPS = [(0, 1), (512, 1)] + [(1024 + 1024 * k, 2) for k in range(15)]


@lru_cache(maxsize=1)
def _build():
    import concourse.bacc as bacc
    import concourse.mybir as mybir
    import concourse.tile as tile

    f32 = mybir.dt.float32
    bf16 = mybir.dt.bfloat16
    f16 = mybir.dt.float16

    nc = bacc.Bacc("TRN2", target_bir_lowering=False, debug=False, num_devices=N_CORES)

    xp9a = nc.dram_tensor("xp9a", [128, NS * RCOL], bf16, kind="ExternalInput")
    xp9b = nc.dram_tensor("xp9b", [3, NCOLS], bf16, kind="ExternalInput")
    wb = nc.dram_tensor("wb", [KP, NOUT], bf16, kind="ExternalInput")
    y = nc.dram_tensor("y", [NOUT, H * W], f16, kind="ExternalOutput")

    with tile.TileContext(nc) as tc:
        with (
            tc.tile_pool(name="xin", bufs=1) as xin_pool,
            tc.tile_pool(name="stag", bufs=1) as stag_pool,
            tc.tile_pool(name="yout", bufs=1) as y_pool,
            tc.tile_pool(name="consts", bufs=1) as c_pool,
            tc.tile_pool(name="cv_ps", bufs=3, space="PSUM") as cv_psum,
        ):
            wb_t = c_pool.tile([KP, NOUT], bf16, tag="wb")
            nc.sync.dma_start(out=wb_t[:, :], in_=wb.ap()[:, :])

            stagA = stag_pool.tile([128, NS * RCOL], bf16, tag="stagA")
            nc.scalar.dma_start(out=stagA[:, :], in_=xp9a.ap()[:, :])

            xin = xin_pool.tile([KP, NCOLS], bf16, tag="xin")
            # K-rows 16-18 (last 2 taps + ones): scalar HWDGE loads after
            # stagA.  Keeping gpsimd quiet here avoids SWDGE descriptor-
            # ring traffic stealing SDMA engines 7/15 from the stagA load.
            for s in range(NS):
                c0 = s * SLICE_COLS
                nc.scalar.dma_start(
                    out=xin[16:19, c0 : c0 + SLICE_COLS],
                    in_=xp9b.ap()[:, c0 : c0 + SLICE_COLS],
                )
            # warm-up matmuls: ramp the PE clock while inputs land; they
            # scribble on cv-pool tiles that real matmuls overwrite
            wtiles = [
                cv_psum.tile([NOUT, 1024], f32, tag="cv", name=f"warm{i}")
                for i in range(3)
            ]
            for wi in range(NWARM):
                nc.tensor.matmul(
                    wtiles[wi % 3][:, 0:128],
                    wb_t[:, :],
                    wb_t[:, :],
                    start=True,
                    stop=True,
                )
            # K-rows 0-15: per-slice SBUF->SBUF reshape from stagA
            for s in range(NS):
                nc.sync.dma_start(
                    out=xin[0:16, s * SLICE_COLS : (s + 1) * SLICE_COLS],
                    in_=stagA[:, s * RCOL : (s + 1) * RCOL],
                )

            yt = y_pool.tile([NOUT, NCOLS], f16, tag="yt")

            def cp_vec(dst, src):
                nc.vector.tensor_copy(dst, src)

            def cp_act(dst, src):
                nc.scalar.copy(dst, src)

            cpe = [cp_vec, cp_act]
            dmae = [nc.sync, nc.gpsimd]
            for j, (c0, nb) in enumerate(_GROUPS):
                gcols = nb * CHW
                if nb == 2:
                    ps = cv_psum.tile([NOUT, 1024], f32, tag="cv", name=f"cv{j}")
                else:
                    ps = cv_psum.tile(
                        [NOUT, 512], f32, tag="cv1", name=f"cv1_{j}", bufs=2
                    )
                for h in range(nb):
                    u0 = c0 + h * CHW
                    nc.tensor.matmul(
                        ps[:, h * CHW : (h + 1) * CHW],
                        wb_t[:, :],
                        xin[:, u0 : u0 + CHW],
                        start=True,
                        stop=True,
                    )
                cpe[j % 2](yt[:, c0 : c0 + gcols], ps[:, :])
                dmae[j % 2].dma_start(
                    out=y.ap()[:, c0 : c0 + gcols],
                    in_=yt[:, c0 : c0 + gcols],
                )

    nc.compile()
    return nc


def _host_prep(x, weight, bias):
    bf = ml_dtypes.bfloat16
    wsum = weight.sum(axis=1)  # [COUT, 3, 3]
    wb = np.zeros((KP, NOUT), np.float32)
    for b in range(BPC):
        for di in range(3):
            for dj in range(3):
                wb[b * 9 + di * 3 + dj, b * COUT : (b + 1) * COUT] = wsum[
                    :, 2 - di, 2 - dj
                ]
    wb[KP - 1, :] = np.tile(bias, BPC)
    wb = wb.astype(bf)

    in_maps = []
    for r in range(N_CORES):
        xs = x[r * BPC : (r + 1) * BPC].sum(axis=1)  # [BPC, H, W] fp32
        xpad = np.zeros((BPC, H + 2, W + 2), np.float32)
        xpad[:, 1 : H + 1, 1 : W + 1] = xs
        xpad = xpad.astype(bf)
        xp9 = np.empty((KP, NCOLS), bf)
        for di in range(3):
            for dj in range(3):
                m = di * 3 + dj
                win = xpad[:, di : di + H, dj : dj + W]  # [BPC, H, W]
                for b in range(BPC):
                    xp9[b * 9 + m] = win[b].reshape(NCOLS)
        xp9[KP - 1] = np.ones((NCOLS,), np.float32).astype(bf)
        # K-rows 0-15 packed for the [128, 2048] spread-load + per-slice
        # on-chip reshape: xp9a[8p+g, s*512+c] = xp9[p, s*4096+g*512+c]
        xp9a = np.ascontiguousarray(
            xp9[0:16].reshape(16, NS, 8, RCOL).transpose(0, 2, 1, 3)
        ).reshape(128, NS * RCOL)
        xp9b = np.ascontiguousarray(xp9[16:19])
        in_maps.append({"xp9a": xp9a, "xp9b": xp9b, "wb": wb})
    return in_maps


def kernel(x, weight, bias):
    from concourse.bass_utils import run_bass_kernel_spmd

    x = np.asarray(x, dtype=np.float32)
    weight = np.asarray(weight, dtype=np.float32)
    bias = np.asarray(bias, dtype=np.float32)
    nc = _build()
    in_maps = _host_prep(x, weight, bias)
    res = run_bass_kernel_spmd(nc, in_maps, core_ids=list(range(N_CORES)))
    out = np.concatenate(
        [
            np.asarray(res.results[r]["y"]).reshape(BPC, COUT, H, W)
            for r in range(N_CORES)
        ],
        axis=0,
    )
    return out.astype(np.float32)
